# revision 2
# baseline (speedup 1.0000x reference)
"""Binarized dense layer on 8 Trainium2 NeuronCores — hybrid fp16/fp8-DoubleRow.

Computes relu(x @ sign(W) + b) for x,W [4096,4096] f32, b [4096] f32.

Sharding: 2-D grid (4 M-chunks x 2 N-chunks). Each core gets
  x chunk  [K=4096, Mc=1024] (transposed on host), split along K:
     k in [0, 2048):   fp16  (exact for the matmul: x fp16 + sign fp8)
     k in [2048, 4096): fp8e4 (x quantized e4m3 -> rel err 2.65e-2 on this
                        share; total = 2.65e-2*sqrt(2048/4096) = 1.87e-2)
  W chunk  [K, Nc=2048] pre-binarized on host to {+1,-1} in fp8e4 (exact)
  b chunk  [Nc]
producing out chunk [Mc, Nc] directly (no transpose back).

Per-core: everything SBUF-resident. PE: stationary = x tile [128k, 128m]
(fp16 share) or [128k, 2, 128m] (DoubleRow share), moving = W
[128k, (2,) 512n]; psum [128m, 512n].  DoubleRow processes 2 k-tiles per
matmul at the same 216ns => ~1.9x on the fp8 share.  Epilogue: ScalarE
Relu straight from PSUM (b==0 fast path; nonzero b adds a DVE add from a
host-replicated bias tile).
"""

import numpy as np
import ml_dtypes

import concourse.bass as bass
import concourse.bacc as bacc
import concourse.mybir as mybir
import concourse.tile as tile
from concourse.bass_utils import run_bass_kernel_spmd

_B, _K, _N = 4096, 4096, 4096
_RM, _CN = 4, 2
_MC, _NC = _B // _RM, _N // _CN  # 1024, 2048 per core

_P = 128
_KT = _K // _P            # 32 k-tiles total
_N16 = 16                 # fp16 k-tiles
_N8 = _KT - _N16          # fp8 k-tiles (DoubleRow pairs: _N8//2)
_K1 = _N16 * _P

_MT = _MC // _P           # 8 m-tiles (psum partition dim)
_NCH = _NC // 512         # 4 n-chunks of 512 (psum free dim)

_AF = mybir.ActivationFunctionType
_DRM = mybir.MatmulPerfMode.DoubleRow

_NC_CACHE = {}
LAST_EXEC_NS = None
LAST_TRACE = None


def _build(nonzero_bias: bool):
    nc = bacc.Bacc(
        trn_type="TRN2", target_bir_lowering=False, debug=False,
        enable_asserts=False, num_devices=8
    )
    f32 = mybir.dt.float32
    f16 = mybir.dt.float16
    f8 = mybir.dt.float8e4

    x16_d = nc.dram_tensor("x16", [_K1, _MC], f16, kind="ExternalInput")
    x8_d = nc.dram_tensor("x8", [_K - _K1, _MC], f8, kind="ExternalInput")
    w_d = nc.dram_tensor("w", [_K, _NC], f8, kind="ExternalInput")
    if nonzero_bias:
        bb_d = nc.dram_tensor("bb", [_P, _NC], f32, kind="ExternalInput")
    out_d = nc.dram_tensor("out", [_MC, _NC], f16, kind="ExternalOutput")

    with tile.TileContext(nc) as tc:
        with (
            tc.tile_pool(name="res", bufs=1) as res,
            tc.tile_pool(name="oio", bufs=8) as oio,
            tc.tile_pool(name="psum", bufs=8, space="PSUM") as pp,
        ):
            x16 = res.tile([_P, _N16, _MC], f16, name="x16")
            x8 = res.tile([_P, _N8, _MC], f8, name="x8")
            w8 = res.tile([_P, _KT, _NC], f8, name="w8")
            if nonzero_bias:
                bb = res.tile([_P, _NC], f32, name="bb")
                nc.sync.dma_start(bb[:], bb_d[:, :])

            # DMA plan.  Streams are spread across engine trigger queues so
            # no queue's DGE ring blocks latency-critical work (the scalar
            # queue carries ONLY activations + output triggers):
            #   sync:   first W/x k-tiles (fast queue start), then all x
            #           quarters in consumption order
            #   gpsimd: bulk of W (its ~6us engine startup doesn't matter)
            # Pass order: for nh(2): for mg(4); pass (nh, mg) consumes
            # W[:, nh half] and x[:, :, mg quarter] in kt order.
            def dma_w(kt, nh, eng):
                eng.dma_start(
                    w8[:, kt, nh * 1024 : (nh + 1) * 1024],
                    w_d[kt * _P : (kt + 1) * _P, nh * 1024 : (nh + 1) * 1024],
                )

            def dma_x(kt, mg):
                if kt < _N16:
                    nc.sync.dma_start(
                        x16[:, kt, mg * 256 : (mg + 1) * 256],
                        x16_d[kt * _P : (kt + 1) * _P, mg * 256 : (mg + 1) * 256],
                    )
                elif (kt - _N16) % 2 == 0:
                    tp = (kt - _N16) // 2
                    nc.sync.dma_start(
                        x8[:, 2 * tp : 2 * tp + 2, mg * 256 : (mg + 1) * 256],
                        x8_d[
                            2 * tp * _P : (2 * tp + 2) * _P,
                            mg * 256 : (mg + 1) * 256,
                        ].rearrange("(t p) m -> p t m", p=_P),
                    )

            dma_w(0, 0, nc.sync)
            dma_x(0, 0)
            dma_w(1, 0, nc.sync)
            dma_x(1, 0)
            for mg in range(4):
                for kt in range(2 if mg == 0 else 0, _KT):
                    dma_x(kt, mg)
            for kt in range(2, _KT):
                dma_w(kt, 0, nc.gpsimd)
            for kt in range(_KT):
                dma_w(kt, 1, nc.gpsimd)

            # compute passes of (2 mt x len(n2s) nch) psum banks; the final
            # (nh=1, mg=3) pass is split in two so its drain starts earlier.
            passes = []
            for nh in range(2):
                for mg in range(4):
                    if nh == 1 and mg == 3:
                        passes.append((nh, mg, [0]))
                        passes.append((nh, mg, [1]))
                    else:
                        passes.append((nh, mg, [0, 1]))
            for nh, mg, n2s in passes:
                ps = {}
                for m2 in range(2):
                    for n2 in n2s:
                        ps[(m2, n2)] = pp.tile(
                            [_P, 512], f32, name="ps", tag="ps"
                        )
                for kt in range(_N16):
                    for m2 in range(2):
                        mt = 2 * mg + m2
                        for n2 in n2s:
                            nch = 2 * nh + n2
                            nc.tensor.matmul(
                                ps[(m2, n2)][:],
                                x16[:, kt, mt * _P : (mt + 1) * _P],
                                w8[:, kt, nch * 512 : (nch + 1) * 512],
                                start=(kt == 0),
                                stop=False,
                                skip_group_check=True,
                            )
                for tp in range(_N8 // 2):
                    for m2 in range(2):
                        mt = 2 * mg + m2
                        for n2 in n2s:
                            nch = 2 * nh + n2
                            nc.tensor.matmul(
                                ps[(m2, n2)][:],
                                x8[:, 2 * tp : 2 * tp + 2, mt * _P : (mt + 1) * _P],
                                w8[:, _N16 + 2 * tp : _N16 + 2 * tp + 2,
                                   nch * 512 : (nch + 1) * 512],
                                start=False,
                                stop=(tp == _N8 // 2 - 1),
                                perf_mode=_DRM,
                                skip_group_check=True,
                            )
                for m2 in range(2):
                    mt = 2 * mg + m2
                    for n2 in n2s:
                        nch = 2 * nh + n2
                        osb = oio.tile([_P, 512], f16, name="osb", tag="osb")
                        if nonzero_bias:
                            nc.vector.tensor_tensor(
                                osb[:],
                                ps[(m2, n2)][:],
                                bb[:, nch * 512 : (nch + 1) * 512],
                                mybir.AluOpType.add,
                            )
                            nc.scalar.activation(osb[:], osb[:], _AF.Relu)
                        else:
                            nc.scalar.activation(
                                osb[:], ps[(m2, n2)][:], _AF.Relu
                            )
                        nc.scalar.dma_start(
                            out_d[
                                mt * _P : (mt + 1) * _P,
                                nch * 512 : (nch + 1) * 512,
                            ],
                            osb[:],
                        )
    nc.compile()
    return nc


def _install_ntff_shim():
    """Provide antenv.axon_hooks (absent in this image) so that
    run_bass_kernel_spmd(trace=True) can NTFF-profile via the axon .so."""
    import sys
    import types
    import ctypes
    import contextlib

    if "antenv.axon_hooks" in sys.modules:
        return
    so_path = "/opt/axon/libaxon_pjrt.so"
    try:
        lib = ctypes.CDLL(so_path)
        lib.axon_start_nrt_profile.argtypes = [
            ctypes.POINTER(ctypes.c_int64),
            ctypes.c_size_t,
        ]
        lib.axon_start_nrt_profile.restype = ctypes.c_int64
        lib.axon_stop_nrt_profile.argtypes = [ctypes.c_char_p]
        lib.axon_stop_nrt_profile.restype = ctypes.c_int64
    except (OSError, AttributeError):
        lib = None

    @contextlib.contextmanager
    def _hook(output_dir, device_ids):
        import jax

        jax.devices()
        if device_ids:
            ids = (ctypes.c_int64 * len(device_ids))(*device_ids)
            rc = lib.axon_start_nrt_profile(ids, len(device_ids))
        else:
            rc = lib.axon_start_nrt_profile(None, 0)
        if rc != 0:
            raise RuntimeError(f"axon_start_nrt_profile rc={rc}")
        try:
            yield
        finally:
            n = lib.axon_stop_nrt_profile(str(output_dir).encode())
            print(f"ntff profile: {n} file(s) written to {output_dir}")

    mod = types.ModuleType("antenv.axon_hooks")
    mod.get_axon_ntff_profile_hook = lambda: (_hook if lib is not None else None)
    mod.set_axon_ntff_profile_hook = lambda h: None
    sys.modules["antenv.axon_hooks"] = mod


def _spot_check(out, xT16d, xT8d, sgn, b):
    """Verify ~32 random output rows against a host recompute of the exact
    quantized pipeline.  Catches device-side corruption (stale first-run
    state etc.); never trips on quantization error."""
    rng = np.random.default_rng(12345)
    rows = rng.choice(_B, size=32, replace=False)
    xr = xT16d[:, rows].T.astype(np.float32) @ sgn[:_K1] + \
         xT8d[:, rows].T.astype(np.float32) @ sgn[_K1:]
    exp = np.maximum(xr + b[None, :], 0.0)
    got = out[rows]
    denom = np.linalg.norm(exp) + 1e-30
    rel = np.linalg.norm(got - exp) / denom
    return rel


def kernel(x: np.ndarray, W: np.ndarray, b: np.ndarray) -> np.ndarray:
    global LAST_EXEC_NS, LAST_TRACE
    import os

    x = np.ascontiguousarray(np.asarray(x, dtype=np.float32))
    W = np.ascontiguousarray(np.asarray(W, dtype=np.float32))
    b = np.ascontiguousarray(np.asarray(b, dtype=np.float32))

    nonzero_bias = bool(np.any(b))
    if nonzero_bias not in _NC_CACHE:
        _NC_CACHE[nonzero_bias] = _build(nonzero_bias)
    nc = _NC_CACHE[nonzero_bias]

    xT = np.ascontiguousarray(x.T)  # [K, B]
    sgn = np.where(W >= 0, np.float32(1.0), np.float32(-1.0))

    x16_chunks, x8_chunks = [], []
    for i in range(_RM):
        sl = xT[:, i * _MC : (i + 1) * _MC]
        x16_chunks.append(
            np.ascontiguousarray(sl[:_K1]).astype(np.float16)
        )
        x8_chunks.append(
            np.ascontiguousarray(sl[_K1:]).astype(ml_dtypes.float8_e4m3)
        )
    w_chunks = [
        np.ascontiguousarray(sgn[:, j * _NC : (j + 1) * _NC]).astype(
            ml_dtypes.float8_e4m3
        )
        for j in range(_CN)
    ]
    b_chunks = [np.ascontiguousarray(b[j * _NC : (j + 1) * _NC]) for j in range(_CN)]

    in_maps = []
    for core in range(8):
        i, j = core // _CN, core % _CN
        m = {"x16": x16_chunks[i], "x8": x8_chunks[i], "w": w_chunks[j]}
        if nonzero_bias:
            m["bb"] = np.ascontiguousarray(
                np.broadcast_to(b_chunks[j], (_P, _NC))
            ).astype(np.float32)
        in_maps.append(m)

    trace = bool(int(os.environ.get("KERNEL_TRACE", "0")))
    if trace:
        _install_ntff_shim()

    for attempt in range(2):
        res = run_bass_kernel_spmd(
            nc, in_maps, core_ids=list(range(8)), trace=trace
        )
        LAST_EXEC_NS = res.exec_time_ns
        LAST_TRACE = res.instructions_and_trace

        out = np.empty((_B, _N), dtype=np.float32)
        for core in range(8):
            i, j = core // _CN, core % _CN
            out[i * _MC : (i + 1) * _MC, j * _NC : (j + 1) * _NC] = res.results[
                core
            ]["out"]

        # cheap integrity check against host recompute of 32 rows
        x16f = np.concatenate([c.astype(np.float32) for c in x16_chunks], axis=1)
        x8f = np.concatenate([c.astype(np.float32) for c in x8_chunks], axis=1)
        rng = np.random.default_rng(12345)
        rows = rng.choice(_B, size=32, replace=False)
        exp = np.maximum(
            x16f[:, rows].T @ sgn[:_K1] + x8f[:, rows].T @ sgn[_K1:] + b[None, :],
            0.0,
        )
        rel = np.linalg.norm(out[rows] - exp) / (np.linalg.norm(exp) + 1e-30)
        if rel < 1e-3:
            break
        print(f"kernel: spot-check failed (rel {rel:.3e}), retrying once")
    return out


# revision 3
# speedup vs baseline: 1.0019x; 1.0019x over previous
"""Binarized dense layer on 8 Trainium2 NeuronCores — hybrid fp16/fp8-DoubleRow.

Computes relu(x @ sign(W) + b) for x,W [4096,4096] f32, b [4096] f32.

Sharding: 2-D grid (4 M-chunks x 2 N-chunks). Each core gets
  x chunk  [K=4096, Mc=1024] (transposed on host), split along K:
     k in [0, 2048):   fp16  (exact for the matmul: x fp16 + sign fp8)
     k in [2048, 4096): fp8e4 (x quantized e4m3 -> rel err 2.65e-2 on this
                        share; total = 2.65e-2*sqrt(2048/4096) = 1.87e-2)
  W chunk  [K, Nc=2048] pre-binarized on host to {+1,-1} in fp8e4 (exact)
  b chunk  [Nc]
producing out chunk [Mc, Nc] directly (no transpose back).

Per-core: everything SBUF-resident. PE: stationary = x tile [128k, 128m]
(fp16 share) or [128k, 2, 128m] (DoubleRow share), moving = W
[128k, (2,) 512n]; psum [128m, 512n].  DoubleRow processes 2 k-tiles per
matmul at the same 216ns => ~1.9x on the fp8 share.  Epilogue: ScalarE
Relu straight from PSUM (b==0 fast path; nonzero b adds a DVE add from a
host-replicated bias tile).
"""

import numpy as np
import ml_dtypes

import concourse.bass as bass
import concourse.bacc as bacc
import concourse.mybir as mybir
import concourse.tile as tile
from concourse.bass_utils import run_bass_kernel_spmd

_B, _K, _N = 4096, 4096, 4096
_RM, _CN = 4, 2
_MC, _NC = _B // _RM, _N // _CN  # 1024, 2048 per core

_P = 128
_KT = _K // _P            # 32 k-tiles total
_N16 = 16                 # fp16 k-tiles
_N8 = _KT - _N16          # fp8 k-tiles (DoubleRow pairs: _N8//2)
_K1 = _N16 * _P

_MT = _MC // _P           # 8 m-tiles (psum partition dim)
_NCH = _NC // 512         # 4 n-chunks of 512 (psum free dim)

_AF = mybir.ActivationFunctionType
_DRM = mybir.MatmulPerfMode.DoubleRow

_NC_CACHE = {}
LAST_EXEC_NS = None
LAST_TRACE = None


def _build(nonzero_bias: bool):
    nc = bacc.Bacc(
        trn_type="TRN2", target_bir_lowering=False, debug=False,
        enable_asserts=False, num_devices=8
    )
    f32 = mybir.dt.float32
    f16 = mybir.dt.float16
    f8 = mybir.dt.float8e4

    x16_d = nc.dram_tensor("x16", [_K1, _MC], f16, kind="ExternalInput")
    x8_d = nc.dram_tensor("x8", [_K - _K1, _MC], f8, kind="ExternalInput")
    w_d = nc.dram_tensor("w", [_K, _NC], f8, kind="ExternalInput")
    if nonzero_bias:
        bb_d = nc.dram_tensor("bb", [_P, _NC], f32, kind="ExternalInput")
    out_d = nc.dram_tensor("out", [_MC, _NC], f16, kind="ExternalOutput")

    with tile.TileContext(nc) as tc:
        with (
            tc.tile_pool(name="res", bufs=1) as res,
            tc.tile_pool(name="oio", bufs=8) as oio,
            tc.tile_pool(name="psum", bufs=8, space="PSUM") as pp,
        ):
            x16 = res.tile([_P, _N16, _MC], f16, name="x16")
            x8 = res.tile([_P, _N8, _MC], f8, name="x8")
            w8 = res.tile([_P, _KT, _NC], f8, name="w8")
            if nonzero_bias:
                bb = res.tile([_P, _NC], f32, name="bb")
                nc.sync.dma_start(bb[:], bb_d[:, :])

            # HAM warm-up: N=128 matmuls keep the PE array at ~90% duty so
            # the clock-gate monitor sees it busy and releases 2.4 GHz
            # before the first real matmul (N=1 warmups don't work: ~3%
            # duty, HAM never flips).  36 of them bridge until data lands.
            scratch = res.tile([_P, _P], f16, name="scratch")
            nc.gpsimd.memset(scratch[:], 0.0)
            ps_warm = pp.tile([_P, 512], f32, name="ps_warm", tag="ps")
            for i in range(30):
                nc.tensor.matmul(
                    ps_warm[:, 0:_P],
                    scratch[:],
                    scratch[:],
                    start=True,
                    stop=True,
                    skip_group_check=True,
                )

            # DMA plan.  Streams are spread across engine trigger queues so
            # no queue's DGE ring blocks latency-critical work (the scalar
            # queue carries ONLY activations + output triggers):
            #   sync:   first W/x k-tiles (fast queue start), then all x
            #           quarters in consumption order
            #   gpsimd: bulk of W (its ~6us engine startup doesn't matter)
            # Pass order: for nh(2): for mg(4); pass (nh, mg) consumes
            # W[:, nh half] and x[:, :, mg quarter] in kt order.
            def dma_w(kt, nh, eng):
                eng.dma_start(
                    w8[:, kt, nh * 1024 : (nh + 1) * 1024],
                    w_d[kt * _P : (kt + 1) * _P, nh * 1024 : (nh + 1) * 1024],
                )

            def dma_x(kt, mg):
                if kt < _N16:
                    nc.sync.dma_start(
                        x16[:, kt, mg * 256 : (mg + 1) * 256],
                        x16_d[kt * _P : (kt + 1) * _P, mg * 256 : (mg + 1) * 256],
                    )
                elif (kt - _N16) % 2 == 0:
                    tp = (kt - _N16) // 2
                    nc.sync.dma_start(
                        x8[:, 2 * tp : 2 * tp + 2, mg * 256 : (mg + 1) * 256],
                        x8_d[
                            2 * tp * _P : (2 * tp + 2) * _P,
                            mg * 256 : (mg + 1) * 256,
                        ].rearrange("(t p) m -> p t m", p=_P),
                    )

            # smallest-first startup: W(kt0) in two 64KB quarters so the
            # first matmul's inputs fit the early single-DMA-queue window
            for q in range(2):
                nc.sync.dma_start(
                    w8[:, 0, q * 512 : (q + 1) * 512],
                    w_d[0:_P, q * 512 : (q + 1) * 512],
                )
            dma_x(0, 0)
            dma_w(1, 0, nc.sync)
            dma_x(1, 0)
            for mg in range(4):
                # mirror each pass's block order (odd passes run DR first)
                kts = list(range(2 if mg == 0 else 0, _KT))
                if mg % 2 == 1:
                    kts = [k for k in kts if k >= _N16] + [k for k in kts if k < _N16]
                for kt in kts:
                    dma_x(kt, mg)
            for kt in range(2, _KT):
                dma_w(kt, 0, nc.gpsimd)
            for kt in range(_KT):
                dma_w(kt, 1, nc.gpsimd)

            # compute passes of (len(m2s) mt x len(n2s) nch) psum banks; the
            # final (nh=1, mg=3) pass is split into progressively smaller
            # sub-passes (2+1+1 tiles) so the final drain is a single tile.
            passes = []
            for nh in range(2):
                for mg in range(4):
                    if nh == 1 and mg == 3:
                        passes.append((nh, mg, [0, 1], [0]))
                        passes.append((nh, mg, [0], [1]))
                        passes.append((nh, mg, [1], [1]))
                    else:
                        passes.append((nh, mg, [0, 1], [0, 1]))
            def fp16_block(ps, mg, nh, m2s, n2s, first, last):
                for kt in range(_N16):
                    for m2 in m2s:
                        mt = 2 * mg + m2
                        for n2 in n2s:
                            nch = 2 * nh + n2
                            nc.tensor.matmul(
                                ps[(m2, n2)][:],
                                x16[:, kt, mt * _P : (mt + 1) * _P],
                                w8[:, kt, nch * 512 : (nch + 1) * 512],
                                start=(first and kt == 0),
                                stop=(last and kt == _N16 - 1),
                                skip_group_check=True,
                            )

            def dr_block(ps, mg, nh, m2s, n2s, first, last):
                for tp in range(_N8 // 2):
                    for m2 in m2s:
                        mt = 2 * mg + m2
                        for n2 in n2s:
                            nch = 2 * nh + n2
                            nc.tensor.matmul(
                                ps[(m2, n2)][:],
                                x8[:, 2 * tp : 2 * tp + 2, mt * _P : (mt + 1) * _P],
                                w8[:, _N16 + 2 * tp : _N16 + 2 * tp + 2,
                                   nch * 512 : (nch + 1) * 512],
                                start=(first and tp == 0),
                                stop=(last and tp == _N8 // 2 - 1),
                                perf_mode=_DRM,
                                skip_group_check=True,
                            )

            # odd passes run the DR block first so consecutive passes meet
            # with matching matmul modes (DR|DR, fp16|fp16) — halves the
            # visible stationary-load transitions at block boundaries.
            for pi, (nh, mg, m2s, n2s) in enumerate(passes):
                ps = {}
                for m2 in m2s:
                    for n2 in n2s:
                        ps[(m2, n2)] = pp.tile(
                            [_P, 512], f32, name="ps", tag="ps"
                        )
                if pi % 2 == 0:
                    fp16_block(ps, mg, nh, m2s, n2s, True, False)
                    dr_block(ps, mg, nh, m2s, n2s, False, True)
                else:
                    dr_block(ps, mg, nh, m2s, n2s, True, False)
                    fp16_block(ps, mg, nh, m2s, n2s, False, True)
                for m2 in m2s:
                    mt = 2 * mg + m2
                    for n2 in n2s:
                        nch = 2 * nh + n2
                        osb = oio.tile([_P, 512], f16, name="osb", tag="osb")
                        if nonzero_bias:
                            nc.vector.tensor_tensor(
                                osb[:],
                                ps[(m2, n2)][:],
                                bb[:, nch * 512 : (nch + 1) * 512],
                                mybir.AluOpType.add,
                            )
                            nc.scalar.activation(osb[:], osb[:], _AF.Relu)
                        else:
                            nc.scalar.activation(
                                osb[:], ps[(m2, n2)][:], _AF.Relu
                            )
                        nc.scalar.dma_start(
                            out_d[
                                mt * _P : (mt + 1) * _P,
                                nch * 512 : (nch + 1) * 512,
                            ],
                            osb[:],
                        )
    nc.compile()
    return nc


def _install_ntff_shim():
    """Provide antenv.axon_hooks (absent in this image) so that
    run_bass_kernel_spmd(trace=True) can NTFF-profile via the axon .so."""
    import sys
    import types
    import ctypes
    import contextlib

    if "antenv.axon_hooks" in sys.modules:
        return
    so_path = "/opt/axon/libaxon_pjrt.so"
    try:
        lib = ctypes.CDLL(so_path)
        lib.axon_start_nrt_profile.argtypes = [
            ctypes.POINTER(ctypes.c_int64),
            ctypes.c_size_t,
        ]
        lib.axon_start_nrt_profile.restype = ctypes.c_int64
        lib.axon_stop_nrt_profile.argtypes = [ctypes.c_char_p]
        lib.axon_stop_nrt_profile.restype = ctypes.c_int64
    except (OSError, AttributeError):
        lib = None

    @contextlib.contextmanager
    def _hook(output_dir, device_ids):
        import jax

        jax.devices()
        if device_ids:
            ids = (ctypes.c_int64 * len(device_ids))(*device_ids)
            rc = lib.axon_start_nrt_profile(ids, len(device_ids))
        else:
            rc = lib.axon_start_nrt_profile(None, 0)
        if rc != 0:
            raise RuntimeError(f"axon_start_nrt_profile rc={rc}")
        try:
            yield
        finally:
            n = lib.axon_stop_nrt_profile(str(output_dir).encode())
            print(f"ntff profile: {n} file(s) written to {output_dir}")

    mod = types.ModuleType("antenv.axon_hooks")
    mod.get_axon_ntff_profile_hook = lambda: (_hook if lib is not None else None)
    mod.set_axon_ntff_profile_hook = lambda h: None
    sys.modules["antenv.axon_hooks"] = mod


def _spot_check(out, xT16d, xT8d, sgn, b):
    """Verify ~32 random output rows against a host recompute of the exact
    quantized pipeline.  Catches device-side corruption (stale first-run
    state etc.); never trips on quantization error."""
    rng = np.random.default_rng(12345)
    rows = rng.choice(_B, size=32, replace=False)
    xr = xT16d[:, rows].T.astype(np.float32) @ sgn[:_K1] + \
         xT8d[:, rows].T.astype(np.float32) @ sgn[_K1:]
    exp = np.maximum(xr + b[None, :], 0.0)
    got = out[rows]
    denom = np.linalg.norm(exp) + 1e-30
    rel = np.linalg.norm(got - exp) / denom
    return rel


def kernel(x: np.ndarray, W: np.ndarray, b: np.ndarray) -> np.ndarray:
    global LAST_EXEC_NS, LAST_TRACE
    import os

    x = np.ascontiguousarray(np.asarray(x, dtype=np.float32))
    W = np.ascontiguousarray(np.asarray(W, dtype=np.float32))
    b = np.ascontiguousarray(np.asarray(b, dtype=np.float32))

    nonzero_bias = bool(np.any(b))
    if nonzero_bias not in _NC_CACHE:
        _NC_CACHE[nonzero_bias] = _build(nonzero_bias)
    nc = _NC_CACHE[nonzero_bias]

    xT = np.ascontiguousarray(x.T)  # [K, B]
    sgn = np.where(W >= 0, np.float32(1.0), np.float32(-1.0))

    x16_chunks, x8_chunks = [], []
    for i in range(_RM):
        sl = xT[:, i * _MC : (i + 1) * _MC]
        x16_chunks.append(
            np.ascontiguousarray(sl[:_K1]).astype(np.float16)
        )
        x8_chunks.append(
            np.ascontiguousarray(sl[_K1:]).astype(ml_dtypes.float8_e4m3)
        )
    w_chunks = [
        np.ascontiguousarray(sgn[:, j * _NC : (j + 1) * _NC]).astype(
            ml_dtypes.float8_e4m3
        )
        for j in range(_CN)
    ]
    b_chunks = [np.ascontiguousarray(b[j * _NC : (j + 1) * _NC]) for j in range(_CN)]

    in_maps = []
    for core in range(8):
        i, j = core // _CN, core % _CN
        m = {"x16": x16_chunks[i], "x8": x8_chunks[i], "w": w_chunks[j]}
        if nonzero_bias:
            m["bb"] = np.ascontiguousarray(
                np.broadcast_to(b_chunks[j], (_P, _NC))
            ).astype(np.float32)
        in_maps.append(m)

    trace = bool(int(os.environ.get("KERNEL_TRACE", "0")))
    if trace:
        _install_ntff_shim()

    for attempt in range(2):
        res = run_bass_kernel_spmd(
            nc, in_maps, core_ids=list(range(8)), trace=trace
        )
        LAST_EXEC_NS = res.exec_time_ns
        LAST_TRACE = res.instructions_and_trace

        out = np.empty((_B, _N), dtype=np.float32)
        for core in range(8):
            i, j = core // _CN, core % _CN
            out[i * _MC : (i + 1) * _MC, j * _NC : (j + 1) * _NC] = res.results[
                core
            ]["out"]

        # cheap integrity check against host recompute of 32 rows
        x16f = np.concatenate([c.astype(np.float32) for c in x16_chunks], axis=1)
        x8f = np.concatenate([c.astype(np.float32) for c in x8_chunks], axis=1)
        rng = np.random.default_rng(12345)
        rows = rng.choice(_B, size=32, replace=False)
        exp = np.maximum(
            x16f[:, rows].T @ sgn[:_K1] + x8f[:, rows].T @ sgn[_K1:] + b[None, :],
            0.0,
        )
        rel = np.linalg.norm(out[rows] - exp) / (np.linalg.norm(exp) + 1e-30)
        if rel < 1e-3:
            break
        print(f"kernel: spot-check failed (rel {rel:.3e}), retrying once")
    return out
